# revision 16
# baseline (speedup 1.0000x reference)
"""Trainium2 Bass kernel for nn_Covar_Attn (MPNCOV-style covariance pooling).

Per sample s (of 32): X = x[s] viewed [C=512, M=784]
  cov  = (X-mu) @ (X-mu)^T / M                  [512, 512]
  A    = cov / trace(cov)
  Ysqrt= Newton-Schulz(A, 5 iters) * sqrt(trace)
  w    = mean over rows of Ysqrt                [512]
  y[s] = w[:, None] * X

Sharding: pure data parallel, 4 samples per NeuronCore across 8 cores.

v2 design:
- All matmul operands bf16 (1 cyc/col at any width; FWL weight loads on HW).
- No centering pass: cov accumulated from raw x with a rank-1 -M*mu*mu^T
  matmul folded into the same PSUM; x stays raw so the final scale is one op.
- W-iteration Newton-Schulz: with W_k = Z_k Y_k (= A Z_k^2),
  W_{k+1} = (9 W - 6 W^2 + W^3)/4 needs 2 matrix products per iteration
  (vs 3 for the Y/Z form); the 9W and -6W^2 terms are accumulated into the
  W^3 PSUM via constant-diagonal matmuls.
- 1^T Ysqrt = 1^T An ZY0 ZY1 ZY2 ZY3 ZY4 with v@ZY_k = 1.5v - 0.5 v@W_k and
  v@W4 expanded into three extra W3 matvecs. The first two steps come free
  from accum_out rowsums of the An and S1 copy-outs; the remaining 6 matvec
  steps run as row-mode chains on the PE.
- All matrices symmetric: only upper-triangle chunk-rows are computed
  (true-triangle widths 512/384/256/128); lower blocks are mirrored with PE
  transposes and destination-contiguous batched copies.
- PSUM->SBUF copies alternate between Activation and DVE (gpsimd has no
  PSUM port); gpsimd takes SBUF-resident work (final scaling, combines).
"""

import numpy as np
from contextlib import ExitStack

import concourse.bass as bass
import concourse.mybir as mybir
import concourse.tile as tile
from concourse import bacc
from concourse.bass_utils import run_bass_kernel_spmd

N_CORES = 8
B, C, H, W = 32, 512, 28, 28
M = H * W            # 784
B_LOC = B // N_CORES  # 4 samples per core
CCH = C // 128       # 4 chunks of 128 rows
MCH = 7              # m chunks
MC = M // MCH        # 112

F32 = mybir.dt.float32
F32R = mybir.dt.float32r
BF16 = mybir.dt.bfloat16
MULT = mybir.AluOpType.mult
ADD = mybir.AluOpType.add
SUB = mybir.AluOpType.subtract
AX = mybir.AxisListType.X
COPYF = mybir.ActivationFunctionType.Copy


def _wid(i):
    # true-triangle width of chunk-row i (diag block + right part)
    return C - i * 128


def _fill_diag(nc, t, val):
    nc.gpsimd.memset(t[:], 0.0)
    nc.gpsimd.affine_select(
        out=t[:],
        in_=t[:],
        compare_op=mybir.AluOpType.not_equal,
        fill=val,
        base=0,
        pattern=[[-1, 128]],
        channel_multiplier=1,
    )


class _Emit:
    def __init__(self, ctx, tc, x_ap, y_ap):
        nc = self.nc = tc.nc
        self.tc = tc
        p = lambda name, bufs, **kw: ctx.enter_context(
            tc.tile_pool(name=name, bufs=bufs, **kw)
        )
        self.consts = p("consts", 1)
        self.xin_p = p("xin", 4)
        self.xt_p = p("xt", 2)
        self.w_p = p("wmat", 8)
        self.s_p = p("smat", 3)
        self.sm_p = p("sm", 2)
        self.ps_mm = p("psmm", 2, space="PSUM")
        self.ps_tr = p("pstr", 2, space="PSUM")
        self.ps_mr = p("psmr", 2, space="PSUM")
        self.ps_sm = p("pssm", 1, space="PSUM")

        # constants
        identf = self.identf = self.consts.tile([128, 128], F32, tag="identf", name="identf")
        _fill_diag(nc, identf, 1.0)
        self.ident_b = self.consts.tile([128, 128], BF16, tag="ident_b", name="ident_b")
        nc.vector.tensor_copy(self.ident_b[:], identf[:])
        d9 = self.consts.tile([128, 128], F32, tag="d9f", name="d9f")
        _fill_diag(nc, d9, 9.0)
        self.diag9 = self.consts.tile([128, 128], BF16, tag="diag9", name="diag9")
        nc.vector.tensor_copy(self.diag9[:], d9[:])
        dm6 = self.consts.tile([128, 128], F32, tag="dm6f", name="dm6f")
        _fill_diag(nc, dm6, -6.0)
        self.diagm6 = self.consts.tile([128, 128], BF16, tag="diagm6", name="diagm6")
        nc.vector.tensor_copy(self.diagm6[:], dm6[:])
        onesf = self.consts.tile([128, 128], F32, tag="onesf", name="onesf")
        nc.gpsimd.memset(onesf[:], 1.0)
        self.ones_b = self.consts.tile([128, 128], BF16, tag="ones_b", name="ones_b")
        nc.vector.tensor_copy(self.ones_b[:], onesf[:])

        self.xr = x_ap.rearrange("b (i p) m -> b p i m", p=128)
        self.yr = y_ap.rearrange("b (i p) m -> b p i m", p=128)
        self.S = [dict() for _ in range(B_LOC)]
        self._cp_i = 0
        self.tail_mode = False

    def _cp(self, out, in_, scale=None):
        """PSUM->SBUF copy alternating between Activation and DVE.

        While tails are in flight (tail_mode), bulk copies go to Activation
        only so the DVE stays free for the latency-critical tail chain."""
        use_act = self.tail_mode or (self._cp_i % 2 == 0)
        self._cp_i += 1
        if use_act:
            if scale is None:
                self.nc.scalar.copy(out, in_)
            else:
                self.nc.scalar.mul(out, in_, scale)
        else:
            if scale is None:
                self.nc.vector.tensor_copy(out, in_)
            else:
                self.nc.vector.tensor_scalar_mul(out, in_, scale)

    # ---------- load & stats ----------
    def dma_in(self, s):
        nc, st = self.nc, self.S[s]
        x_t = st["x"] = self.xin_p.tile([128, CCH, M], F32, tag="x", name="x")
        for i in range(CCH):
            nc.sync.dma_start(x_t[:, i, :], self.xr[s, :, i, :])

    def stats(self, s):
        nc, st = self.nc, self.S[s]
        x_t = st["x"]
        stt = self.sm_p.tile([128, CCH, 2, 6], F32, tag="st", name="st")
        for i in range(CCH):
            for h in range(2):
                nc.vector.bn_stats(
                    stt[:, i, h, :], x_t[:, i, h * (M // 2):(h + 1) * (M // 2)]
                )
        mv = st["mv"] = self.sm_p.tile([128, CCH, 2], F32, tag="mv", bufs=4, name="mv")
        for i in range(CCH):
            nc.vector.bn_aggr(mv[:, i, :], stt[:, i, :, :])
        # trace(cov) = sum_c var_c, broadcast to all partitions via ones-matmul
        var_b = self.sm_p.tile([128, CCH], BF16, tag="var_b", name="var_b")
        nc.gpsimd.tensor_copy(var_b[:], mv[:, :, 1])
        t_ps = self.ps_sm.tile([128, CCH], F32, tag="col", name="sm")
        nc.tensor.matmul(t_ps[:], self.ones_b[:], var_b[:], start=True, stop=True)
        tco = self.sm_p.tile([128, 1], F32, tag="tco", name="tco")
        nc.vector.reduce_sum(out=tco[:], in_=t_ps[:], axis=AX)
        inv = self.sm_p.tile([128, 1], F32, tag="inv", name="inv")
        nc.vector.reciprocal(inv[:], tco[:])
        invM = st["invM"] = self.sm_p.tile([128, 1], F32, tag="invM", bufs=4, name="invM")
        nc.gpsimd.tensor_scalar_mul(invM[:], inv[:], 1.0 / M)
        sq = st["sq"] = self.sm_p.tile([128, 1], F32, tag="sq", bufs=4, name="sq")
        nc.scalar.sqrt(sq[:], tco[:])
        # mu as bf16 rows on partition 0: mupos [1,C] (lhsT slices), -M*mu (rhs)
        mn_ps = self.ps_sm.tile([1, C], F32, tag="row", name="mneg")
        for i in range(CCH):
            nc.tensor.transpose(
                mn_ps[0:1, i * 128:(i + 1) * 128], mv[:, i, 0:1], self.identf[:]
            )
        mupos = st["mupos"] = self.sm_p.tile(
            [1, C], BF16, tag="mupos", bufs=4, name="mupos"
        )
        nc.scalar.copy(mupos[:], mn_ps[:])
        muneg = st["muneg"] = self.sm_p.tile(
            [1, C], BF16, tag="muneg", bufs=4, name="muneg"
        )
        nc.scalar.mul(muneg[:], mn_ps[:], -float(M))

    # ---------- x transposes ----------
    def trans(self, s, j):
        nc, st = self.nc, self.S[s]
        if j == 0:
            st["xt"] = self.xt_p.tile([MC, MCH, C], BF16, tag="xt", name="xt")
        xt, x_t = st["xt"], st["x"]
        tp = self.ps_tr.tile([MC, CCH, 128], F32, tag="tr", name="tr")
        for i in range(CCH):
            nc.tensor.transpose(
                tp[:, i, :], x_t[:, i, j * MC:(j + 1) * MC], self.identf[:]
            )
        self._cp(xt[:, j, :], tp[:, :, :])

    # ---------- symmetric-matrix mirror ----------
    def _mirror(self, mat_t, acc=None):
        """Fill lower blocks of mat_t from the upper triangle.

        For dst chunk k: transpose blocks (i,k), i<k, into one PSUM tile, then
        one contiguous copy into mat_t[:, k, 0:k*128]. With acc, the copies
        run on Activation with accum_out into acc[:, k] (rowsums of the
        mirrored part).
        """
        nc = self.nc
        for k in range(CCH - 1, 0, -1):
            mp = self.ps_mr.tile([128, 3, 128], BF16, tag="mr", name="mr")
            for i in range(k):
                nc.tensor.transpose(
                    mp[:, i, :], mat_t[:, i, k * 128:(k + 1) * 128], self.ident_b[:]
                )
            if acc is not None:
                nc.scalar.activation(
                    mat_t[:, k, 0:k * 128], mp[:, 0:k, :], COPYF,
                    accum_out=acc[:, k:k + 1],
                )
            else:
                self._cp(mat_t[:, k, 0:k * 128], mp[:, 0:k, :])

    # ---------- cov ----------
    def cov(self, s, i):
        nc, st = self.nc, self.S[s]
        if i == 0:
            st["w0"] = self.w_p.tile([128, CCH, C], BF16, tag="W", name="W0")
            st["acc_w0"] = self.sm_p.tile(
                [128, CCH], F32, tag="acc_w0", bufs=4, name="acc_w0"
            )
        xt, w0 = st["xt"], st["w0"]
        w = _wid(i)
        g = self.ps_mm.tile([128, C], F32, tag="mm", name="mm")
        for j in range(MCH):
            nc.tensor.matmul(
                g[:, 0:w], xt[:, j, i * 128:(i + 1) * 128], xt[:, j, C - w:],
                start=(j == 0), stop=False,
            )
        nc.tensor.matmul(
            g[:, 0:w], st["mupos"][0:1, i * 128:(i + 1) * 128],
            st["muneg"][0:1, C - w:],
            start=False, stop=True,
        )
        nc.scalar.activation(
            w0[:, i, C - w:], g[:, 0:w], COPYF,
            scale=st["invM"][:],
            accum_out=st["acc_w0"][:, i:i + 1],
        )

    def cov_mirror(self, s):
        st = self.S[s]
        st["acc_w0m"] = self.sm_p.tile(
            [128, CCH], F32, tag="acc_w0m", bufs=4, name="acc_w0m"
        )
        self._mirror(st["w0"], acc=st["acc_w0m"])

    # ---------- Newton-Schulz W iteration ----------
    def form_S(self, s, i, it):
        """S = W @ W, chunk i."""
        nc, st = self.nc, self.S[s]
        if i == CCH - 1:
            st["s"] = self.s_p.tile([128, CCH, C], BF16, tag="S", name="S")
            if it == 0:
                st["acc_s1"] = self.sm_p.tile(
                    [128, CCH], F32, tag="acc_s1", bufs=4, name="acc_s1"
                )
        wm, sm = st["w"], st["s"]
        w = _wid(i)
        ps = self.ps_mm.tile([128, C], F32, tag="mm", name="mm")
        for k in range(CCH):
            nc.tensor.matmul(
                ps[:, 0:w], wm[:, k, i * 128:(i + 1) * 128], wm[:, k, C - w:],
                start=(k == 0), stop=(k == CCH - 1),
            )
        if it == 0:
            nc.scalar.activation(
                sm[:, i, C - w:], ps[:, 0:w], COPYF,
                accum_out=st["acc_s1"][:, i:i + 1],
            )
        else:
            self._cp(sm[:, i, C - w:], ps[:, 0:w])

    def form_S_mirror(self, s, it):
        st = self.S[s]
        if it == 0:
            st["acc_s1m"] = self.sm_p.tile(
                [128, CCH], F32, tag="acc_s1m", bufs=4, name="acc_s1m"
            )
            self._mirror(st["s"], acc=st["acc_s1m"])
        else:
            self._mirror(st["s"])

    def form_W_pair(self, pair, i):
        """W' = (S @ W - 6 S + 9 W)/4, chunk i, both samples of the pair.

        The S@W parts run per sample; the diag-const matmuls are grouped so
        consecutive PE instructions share the same stationary operand
        (one LdWeights per const per chunk instead of per sample)."""
        nc = self.nc
        pss = {}
        for s in pair:
            st = self.S[s]
            if i == CCH - 1:
                st["wn"] = self.w_p.tile([128, CCH, C], BF16, tag="W", name="Wn")
            wm, sm = st["w"], st["s"]
            w = _wid(i)
            ps = pss[s] = self.ps_mm.tile([128, C], F32, tag="mm", name="mm")
            for k in range(CCH):
                nc.tensor.matmul(
                    ps[:, 0:w], sm[:, k, i * 128:(i + 1) * 128], wm[:, k, C - w:],
                    start=(k == 0), stop=False,
                )
        w = _wid(i)
        for s in pair:
            nc.tensor.matmul(
                pss[s][:, 0:w], self.diagm6[:], self.S[s]["s"][:, i, C - w:],
                start=False, stop=False,
            )
        for s in pair:
            nc.tensor.matmul(
                pss[s][:, 0:w], self.diag9[:], self.S[s]["w"][:, i, C - w:],
                start=False, stop=True,
            )
        for s in pair:
            self._cp(self.S[s]["wn"][:, i, C - w:], pss[s][:, 0:w], scale=0.25)

    def form_W_mirror(self, s):
        self._mirror(self.S[s]["wn"])

    # ---------- tail: row-chain matvecs ----------
    def tail_r2(self, s):
        """r2 = 1.5*rowsum(W0) - 0.5*rowsum(S1) as bf16 column [128, CCH]."""
        nc, st = self.nc, self.S[s]
        rs_w0 = self.sm_p.tile([128, CCH], F32, tag="rs_w0", name="rs_w0")
        nc.gpsimd.tensor_tensor(
            rs_w0[:, 1:], st["acc_w0"][:, 1:], st["acc_w0m"][:, 1:], op=ADD
        )
        nc.gpsimd.tensor_copy(rs_w0[:, 0:1], st["acc_w0"][:, 0:1])
        rs_s1 = self.sm_p.tile([128, CCH], F32, tag="rs_s1", name="rs_s1")
        nc.gpsimd.tensor_tensor(
            rs_s1[:, 1:], st["acc_s1"][:, 1:], st["acc_s1m"][:, 1:], op=ADD
        )
        nc.gpsimd.tensor_copy(rs_s1[:, 0:1], st["acc_s1"][:, 0:1])
        # r2 = 1.5*rs_w0 - 0.5*rs_s1 = 0.5*(3*rs_w0 - rs_s1); v15 = 1.5*r2
        t1 = self.sm_p.tile([128, CCH], F32, tag="t1", name="t1")
        nc.gpsimd.tensor_scalar_mul(t1[:], rs_w0[:], 3.0)
        nc.gpsimd.tensor_tensor(t1[:], t1[:], rs_s1[:], op=SUB)
        vc = self.sm_p.tile([128, CCH], BF16, tag="vc", bufs=4, name="vc")
        nc.gpsimd.tensor_scalar_mul(vc[:], t1[:], 0.5)
        v15 = self.sm_p.tile([128, CCH], F32, tag="v15", bufs=4, name="v15")
        nc.gpsimd.tensor_scalar_mul(v15[:], t1[:], 0.75)
        st["vc"], st["v15"] = vc, v15

    def tail_step(self, s, mat, last=False, save_r4=False, need_v15=True):
        """vc <- 1.5*vc - 0.5*(vc @ mat)."""
        nc, st = self.nc, self.S[s]
        mt = st[mat]
        pr = self.ps_sm.tile([1, C], F32, tag="row", name="row")
        for k in range(CCH):
            nc.tensor.matmul(
                pr[:], st["vc"][:, k:k + 1], mt[:, k, :],
                start=(k == 0), stop=(k == CCH - 1),
            )
        rr = self.sm_p.tile([1, C], BF16, tag="rr", bufs=3, name="rr")
        nc.vector.tensor_scalar_mul(rr[:], pr[:], -0.5)
        tpc = self.ps_sm.tile([128, CCH, 2], BF16, tag="col", name="tpc")
        for k in range(CCH):
            nc.tensor.transpose(
                tpc[:, k, 0:1], rr[0:1, k * 128:(k + 1) * 128],
                self.ident_b[0:1, 0:1],
            )
        if last:
            # fs = (1.5*r4 - 0.5*(v @ mat)) * sq / C
            pre = self.sm_p.tile([128, CCH], F32, tag="pre", name="pre")
            nc.vector.tensor_tensor(pre[:], tpc[:, :, 0], st["v15_r4"][:], op=ADD)
            fs = st["fs"] = self.sm_p.tile([128, CCH], F32, tag="fs", bufs=4, name="fs")
            nc.gpsimd.tensor_scalar(
                fs[:], pre[:], st["sq"][:], 1.0 / C, op0=MULT, op1=MULT
            )
            return
        vn = self.sm_p.tile([128, CCH], BF16, tag="vc", bufs=4, name="vcn")
        nc.vector.tensor_tensor(vn[:], tpc[:, :, 0], st["v15"][:], op=ADD)
        st["vc"] = vn
        if not need_v15:
            return
        if save_r4:
            v15n = self.sm_p.tile([128, CCH], F32, tag="v15r4", bufs=2, name="v15r4")
            st["v15_r4"] = v15n
        else:
            v15n = self.sm_p.tile([128, CCH], F32, tag="v15", bufs=4, name="v15n")
        nc.gpsimd.tensor_scalar_mul(v15n[:], vn[:], 1.5)
        st["v15"] = v15n

    # ---------- final scale & output ----------
    def fin(self, s):
        nc, st = self.nc, self.S[s]
        x_t, fs = st["x"], st["fs"]
        for i in range(CCH):
            eng = (nc.vector, nc.gpsimd, nc.scalar, nc.gpsimd)[i]
            if eng is nc.scalar:
                eng.mul(x_t[:, i, :], x_t[:, i, :], fs[:, i:i + 1])
            else:
                eng.tensor_scalar_mul(x_t[:, i, :], x_t[:, i, :], fs[:, i:i + 1])
            nc.sync.dma_start(self.yr[s, :, i, :], x_t[:, i, :])
        st.clear()

    # ---------- braiding generators ----------
    def gen_transcov(self, pair):
        for s in pair:
            self.stats(s)
        for j in range(MCH):
            for s in pair:
                self.trans(s, j)
            yield
        for i in range(CCH):
            for s in pair:
                self.cov(s, i)
            yield
        for s in pair:
            self.cov_mirror(s)
            self.S[s]["w"] = self.S[s]["w0"]
            yield

    def gen_ns(self, pair):
        for it in range(3):
            for i in range(CCH - 1, -1, -1):
                for s in pair:
                    self.form_S(s, i, it)
                yield
            for s in pair:
                self.form_S_mirror(s, it)
                yield
            for i in range(CCH - 1, -1, -1):
                self.form_W_pair(pair, i)
                yield
            for s in pair:
                self.form_W_mirror(s)
                st = self.S[s]
                st["w"] = st["wn"]
                st["w%d" % (it + 1)] = st["wn"]
                yield

    def gen_tail(self, s):
        self.tail_r2(s)
        yield
        self.tail_step(s, "w1")
        yield
        self.tail_step(s, "w2")
        yield
        self.tail_step(s, "w3", save_r4=True)
        yield
        self.tail_step(s, "w3")
        yield
        self.tail_step(s, "w3", need_v15=False)
        yield
        self.tail_step(s, "w3", last=True)

    @staticmethod
    def _round_robin(gens, strides=None):
        strides = strides or {}
        done = [False] * len(gens)
        while not all(done):
            for gi, g in enumerate(gens):
                for _ in range(strides.get(gi, 1)):
                    if not done[gi]:
                        try:
                            next(g)
                        except StopIteration:
                            done[gi] = True


def _emit(ctx, tc, x_ap, y_ap):
    em = _Emit(ctx, tc, x_ap, y_ap)
    for s in range(B_LOC):
        em.dma_in(s)
    def gen_fins(pair):
        for s in pair:
            em.fin(s)
            yield

    em._round_robin([em.gen_transcov((0, 1))])
    em._round_robin([em.gen_ns((0, 1)), em.gen_transcov((2, 3))])
    em.tail_mode = True
    em._round_robin(
        [em.gen_tail(0), em.gen_tail(1)] + [em.gen_ns((2, 3))] * 1,
        strides={2: 2},
    )
    em.tail_mode = False
    em._round_robin([em.gen_tail(2), em.gen_tail(3), gen_fins((0, 1))])
    em.fin(2)
    em.fin(3)


_NC_CACHE = {}


def _get_nc(reps: int = 1):
    if reps not in _NC_CACHE:
        nc = bacc.Bacc("TRN2", target_bir_lowering=False, debug=False)
        x_ap = nc.dram_tensor("x", [B_LOC, C, M], F32, kind="ExternalInput").ap()
        y_ap = nc.dram_tensor("y", [B_LOC, C, M], F32, kind="ExternalOutput").ap()
        with ExitStack() as ctx:
            tc = ctx.enter_context(tile.TileContext(nc))
            if reps > 1:
                with tc.For_i(0, reps, 1):
                    _emit(ctx, tc, x_ap, y_ap)
            else:
                _emit(ctx, tc, x_ap, y_ap)
        nc.compile()
        _NC_CACHE[reps] = nc
    return _NC_CACHE[reps]


def kernel(x: np.ndarray, _trace: bool = False):
    assert x.shape == (B, C, H, W), x.shape
    xs = np.ascontiguousarray(x.reshape(B, C, M), dtype=np.float32)
    nc = _get_nc()
    in_maps = [
        {"x": np.ascontiguousarray(xs[c * B_LOC:(c + 1) * B_LOC])}
        for c in range(N_CORES)
    ]
    res = run_bass_kernel_spmd(nc, in_maps, core_ids=list(range(N_CORES)), trace=_trace)
    y = np.concatenate([res.results[c]["y"] for c in range(N_CORES)], axis=0)
    out = y.reshape(B, C, H, W).astype(np.float32)
    if _trace:
        return out, res
    return out
